# revision 1
# baseline (speedup 1.0000x reference)
"""HadLinear Trainium2 kernel: per-128-block L2-norm -> Hadamard -> 4-bit
Gaussian-codebook quantize -> rescale -> matmul with w.T/sqrt(128).

Sharding: 8-way data-parallel over tokens (16384 tokens / 8 cores = 2048 each).
Weight host-pre-transposed to [in_dim, out_dim] fp16, replicated per core.

v3 design (all-fp16 dataflow, engine-balanced):
 - Host passes x TRANSPOSED as fp16 [d, tok]; each Hadamard block-row
   [128 feat, 512 tok] DMAs straight into SBUF as the matmul moving operand
   (no PE transposes).
 - Per-(token,block) norms: ACT squares the input macro-tile, a ones[128,128]
   matmul column-sums x^2 AND broadcasts the result across partitions in one
   PE op; ACT Sqrt turns it into s*sqrt(128) broadcast. No DRAM round trip,
   no DVE on the critical path into PE.
 - Quantize staircase fully on DVE in fp16 SBUF using only
   tensor_scalar (4x mode) and tensor_tensor (2x mode); scalar_tensor_tensor
   is avoided entirely (it has no fast DVE modes).
 - Elementwise stages run on [128, 2048] macro-tiles (4 blocks) to amortize
   instruction init overhead; in-place where possible to fit SBUF.
 - Main matmul fp16, group-interleaved with quantization of the next token
   group; PSUM->SBUF output drains on ACT.
"""

import math

import numpy as np

# ---------------------------------------------------------------- constants
BS = 128          # hadamard block size
NLEVELS = 16

_consts = None


def _get_consts():
    global _consts
    if _consts is not None:
        return _consts
    import jax

    _p = (np.arange(NLEVELS) + 0.5) / NLEVELS
    # mirror reference.py exactly (fp32 jax ppf)
    cent = np.asarray(jax.scipy.stats.norm.ppf(_p), dtype=np.float32)
    bound = np.asarray(
        (np.asarray(cent[1:]) + np.asarray(cent[:-1])) * np.float32(0.5),
        dtype=np.float32,
    )
    # positive half: cpos = cent[8:16]; positive boundaries bound[8:15]
    cpos = cent[8:16].copy()
    bpos = bound[8:15].copy()
    dpos = (cpos[1:] - cpos[:-1]).astype(np.float32)  # 7 deltas
    _consts = (cent, bound, cpos, bpos, dpos)
    return _consts


def _hadamard_matrix():
    x = np.eye(BS, dtype=np.float32)
    h = 1
    while h < BS:
        x = x.reshape(BS, -1, 2, h)
        a, b = x[:, :, 0, :], x[:, :, 1, :]
        x = np.concatenate([a + b, a - b], axis=-1)
        h *= 2
    return np.ascontiguousarray(x.reshape(BS, BS))  # out_row = e_i -> M[i, :]


# ---------------------------------------------------------------- builder
def build_module(tok, d, gt, nchunk_n=512, num_devices=8, bpm=4, repeat=1):
    """Build the per-core bass program.

    tok: tokens per core; d: feature dim; gt: tokens per matmul group;
    bpm: 128-blocks per elementwise macro-tile; repeat: emit the whole
    pipeline R times (identical output; for wall-clock diff timing).
    """
    import concourse.bass as bass
    import concourse.tile as tile
    from concourse import bacc, mybir

    f32 = mybir.dt.float32
    f16 = mybir.dt.float16
    A = mybir.AluOpType
    AF = mybir.ActivationFunctionType

    nb = d // BS                    # 128-blocks per row (32)
    nn = d // nchunk_n              # output-col chunks
    nmac = nb // bpm                # macro tiles per group
    # group schedule: small first group to cut pipeline lead-in
    if gt >= 512 and tok % gt == 0 and tok // gt >= 2:
        groups = [128, gt - 128] + [gt] * (tok // gt - 1)
    else:
        groups = [gt] * (tok // gt)
    gstart = list(np.cumsum([0] + groups[:-1]))

    _, _, cpos, bpos, dpos = _get_consts()
    inv_sqrt128 = np.float32(1.0 / math.sqrt(128.0))
    s2_const = np.float32(1.0 / (128.0 * math.sqrt(128.0)))
    # staircase compares run on xh' = |hp| / (s*sqrt(128)); fold sqrt(128)
    # into the boundaries so no reciprocal / extra scale pass is needed.
    bpos_s = [float(np.float32(b) * inv_sqrt128) for b in bpos]

    nc = bacc.Bacc(
        "TRN2", target_bir_lowering=False, debug=False,
        num_devices=num_devices,
    )
    # x transposed on host: [d, tok] fp16
    xt_in = nc.dram_tensor("xt_in", [d, tok], f16, kind="ExternalInput").ap()
    w_t = nc.dram_tensor("w_t", [d, d], f16, kind="ExternalInput").ap()
    hmat_d = nc.dram_tensor("hmat", [BS, BS], f16, kind="ExternalInput").ap()
    out = nc.dram_tensor("out", [tok, d], f16, kind="ExternalOutput").ap()

    xt_v = xt_in.rearrange("(b p) t -> p b t", p=BS)  # [128, nb, tok]
    wt_v = w_t.rearrange("(k p) n -> p k n", p=BS)    # [128, nb, d]

    with tile.TileContext(nc) as tc:
        import contextlib

        ctx = contextlib.ExitStack()
        with ctx:
            singles = ctx.enter_context(tc.tile_pool(name="singles", bufs=1))
            xtc_p = ctx.enter_context(tc.tile_pool(name="xtc", bufs=3))
            sqx_p = ctx.enter_context(tc.tile_pool(name="sqx", bufs=2))
            hpa_p = ctx.enter_context(tc.tile_pool(name="hpa", bufs=2))
            sgn_p = ctx.enter_context(tc.tile_pool(name="sgn", bufs=2))
            ssb_p = ctx.enter_context(tc.tile_pool(name="ssb", bufs=2))
            rec_p = ctx.enter_context(tc.tile_pool(name="rec", bufs=2))
            s2b_p = ctx.enter_context(tc.tile_pool(name="s2b", bufs=2))
            acc_p = ctx.enter_context(tc.tile_pool(name="acc", bufs=2))
            mk_p = ctx.enter_context(tc.tile_pool(name="mk", bufs=2))
            xqg_p = ctx.enter_context(tc.tile_pool(name="xqg", bufs=2))
            w_p = ctx.enter_context(tc.tile_pool(name="wsl", bufs=2))
            ev_p = ctx.enter_context(tc.tile_pool(name="ev", bufs=3))
            hp_p = ctx.enter_context(
                tc.tile_pool(name="hp", bufs=3, space="PSUM"))
            cs_p = ctx.enter_context(
                tc.tile_pool(name="cs", bufs=3, space="PSUM"))
            mp_p = ctx.enter_context(
                tc.tile_pool(name="mp", bufs=2, space="PSUM"))

            hmat_s = singles.tile([BS, BS], f16)
            nc.sync.dma_start(out=hmat_s[:], in_=hmat_d[:, :])
            ones_s = singles.tile([BS, BS], f16)
            nc.vector.memset(ones_s[:], 1.0)

            def quantize_group(g):
                t0, gtg = gstart[g], groups[g]
                mwg = bpm * gtg
                xqg = xqg_p.tile([BS, nb, gtg], f16, tag="xqg")
                for mac in range(nmac):
                    b0 = mac * bpm
                    # load x^T macro block-rows [128 feat, bpm*gtg tok]
                    xtc = xtc_p.tile([BS, bpm, gtg], f16, tag="xtc")
                    nc.sync.dma_start(
                        out=xtc[:], in_=xt_v[:, b0:b0 + bpm, t0:t0 + gtg])
                    xtc2 = xtc[:].rearrange("p b t -> p (b t)")
                    # x^2 for norms (ACT, sbuf->sbuf)
                    sqx = sqx_p.tile([BS, mwg], f16, tag="sqx")
                    nc.scalar.activation(out=sqx[:], in_=xtc2,
                                         func=AF.Square)
                    hpa = hpa_p.tile([BS, mwg], f16, tag="hpa")
                    sgn = sgn_p.tile([BS, mwg], f16, tag="sgn")
                    ssb = ssb_p.tile([BS, mwg], f16, tag="ssb")
                    for j in range(bpm):
                        sl = slice(j * gtg, (j + 1) * gtg)
                        # hadamard (raw): hp = H^T @ x_blk  (fp32 psum)
                        hp = hp_p.tile([BS, gtg], f32, tag="hp")
                        nc.tensor.matmul(hp[:], lhsT=hmat_s[:],
                                         rhs=xtc[:, j, :],
                                         start=True, stop=True)
                        # column-sum bcast: cs[p,t] = sum_f x^2 = s^2
                        cs = cs_p.tile([BS, gtg], f32, tag="cs")
                        nc.tensor.matmul(cs[:], lhsT=ones_s[:],
                                         rhs=sqx[:, sl],
                                         start=True, stop=True)
                        # |hp|, sign(hp), sqrt(cs) -> sbuf fp16 (ACT)
                        nc.scalar.activation(out=hpa[:, sl], in_=hp[:],
                                             func=AF.Abs)
                        nc.scalar.activation(out=sgn[:, sl], in_=hp[:],
                                             func=AF.Sign)
                        # ssb = sqrt(128 * s^2) = s*sqrt(128)
                        nc.scalar.activation(out=ssb[:, sl], in_=cs[:],
                                             func=AF.Sqrt, scale=128.0)
                    # ---- DVE macro stages [128, mwg] ----
                    # s2b = ssb/(128*sqrt(128)) = s/128
                    s2b = s2b_p.tile([BS, mwg], f16, tag="s2b")
                    nc.vector.tensor_scalar_mul(s2b[:], ssb[:],
                                                float(s2_const))
                    # hpa <- xh' = |hp| / (s*sqrt(128)); boundaries pre-scaled
                    rec = rec_p.tile([BS, mwg], f16, tag="rec")
                    with nc.allow_low_precision(
                            reason="1/(s*sqrt(128)) in fp16; ~1/128, fine"):
                        nc.vector.reciprocal(out=rec[:], in_=ssb[:])
                    nc.vector.tensor_mul(hpa[:], hpa[:], rec[:])
                    # sgn <- su = sign * s2
                    nc.vector.tensor_mul(sgn[:], sgn[:], s2b[:])
                    # staircase: acc = sum_i [xh' > b_i'] * d_i
                    acc = acc_p.tile([BS, mwg], f16, tag="acc")
                    nc.vector.tensor_scalar(
                        out=acc[:], in0=hpa[:],
                        scalar1=bpos_s[0], scalar2=float(dpos[0]),
                        op0=A.is_gt, op1=A.mult)
                    for i in range(1, 7):
                        mk = mk_p.tile([BS, mwg], f16, tag="mk")
                        nc.vector.tensor_scalar(
                            out=mk[:], in0=hpa[:],
                            scalar1=bpos_s[i], scalar2=float(dpos[i]),
                            op0=A.is_gt, op1=A.mult)
                        nc.vector.tensor_add(acc[:], acc[:], mk[:])
                    # acc += c0 ; xq = acc * su
                    nc.vector.tensor_scalar_add(acc[:], acc[:],
                                                float(cpos[0]))
                    xq_v = xqg[:, b0:b0 + bpm, :].rearrange(
                        "p b t -> p (b t)")
                    nc.vector.tensor_mul(xq_v, acc[:], sgn[:])
                return xqg

            def matmul_group(g, xqg):
                t0, gtg = gstart[g], groups[g]
                tpg = gtg // 128
                for n in range(nn):
                    wsl = w_p.tile([BS, nb, nchunk_n], f16, tag="wsl")
                    nc.sync.dma_start(
                        out=wsl[:],
                        in_=wt_v[:, :, n * nchunk_n:(n + 1) * nchunk_n])
                    for m in range(tpg):
                        ps = mp_p.tile([BS, nchunk_n], f32, tag="mp")
                        for k in range(nb):
                            nc.tensor.matmul(
                                ps[:],
                                lhsT=xqg[:, k, m * 128:(m + 1) * 128],
                                rhs=wsl[:, k, :],
                                start=(k == 0), stop=(k == nb - 1))
                        ev = ev_p.tile([BS, nchunk_n], f16, tag="ev")
                        nc.scalar.copy(out=ev[:], in_=ps[:])
                        nc.sync.dma_start(
                            out=out[t0 + m * 128:t0 + (m + 1) * 128,
                                    n * nchunk_n:(n + 1) * nchunk_n],
                            in_=ev[:])

            # emission order: Q0 Q1 M0 Q2 M1 ...  (PE overlap)
            ngroup = len(groups)
            for _ in range(repeat):
                pend = []
                pend.append(quantize_group(0))
                for g in range(1, ngroup):
                    pend.append(quantize_group(g))
                    matmul_group(g - 1, pend[g - 1])
                matmul_group(ngroup - 1, pend[ngroup - 1])

    nc.compile()
    return nc


# ---------------------------------------------------------------- driver
_CACHED = None

TOK_FULL = 2048
D_FULL = 4096
GT_FULL = 512


def _get_compiled():
    global _CACHED
    if _CACHED is None:
        from concourse.bass_interp import get_hw_module

        nc = build_module(TOK_FULL, D_FULL, GT_FULL, num_devices=8)
        nc.m = get_hw_module(nc.m)
        _CACHED = nc
    return _CACHED


def _run(input, weight, trace=False):
    from concourse import bass_utils

    nc = _get_compiled()
    x = np.asarray(input, dtype=np.float32).reshape(-1, D_FULL)
    xt = np.ascontiguousarray(x.T.astype(np.float16))  # [d, tok_total]
    wt = np.ascontiguousarray(
        np.asarray(weight, dtype=np.float32).T).astype(np.float16)
    hm = _hadamard_matrix().astype(np.float16)
    ncores = 8
    in_maps = [
        {"xt_in": np.ascontiguousarray(
            xt[:, i * TOK_FULL:(i + 1) * TOK_FULL]),
         "w_t": wt, "hmat": hm}
        for i in range(ncores)
    ]
    res = bass_utils.run_bass_kernel_spmd(
        nc, in_maps, core_ids=list(range(ncores)), trace=trace)
    outs = [res.results[i]["out"] for i in range(ncores)]
    full = np.concatenate(outs, axis=0).astype(np.float32).reshape(input.shape)
    return full, res


def kernel(input, weight):
    out, _ = _run(input, weight, trace=False)
    return out

